# revision 2
# baseline (speedup 1.0000x reference)
"""Trainium2 Bass kernel for nn_BaseEmbedder (retrieval_knn).

Computes, for each of 4096 query embeddings, the 5 nearest db embeddings
(Euclidean), then an inverse-distance-weighted sum of their auxiliary
features.  Runs SPMD on 8 NeuronCores, queries sharded 512/core, db and
aux replicated.

Algorithm per core (512 queries, 4 q-tiles of 128):
  - negS[q, j] = q . x_j - 0.5*|x_j|^2 via a K=33 augmented matmul
    (row 32 of lhsT = ones, row 32 of rhs = -0.5*|x_j|^2).  Top-5
    smallest distances == top-5 largest negS.
  - Per 512-column chunk: TensorE matmul -> PSUM; ScalarE copies the
    chunk to an SBUF f32 buffer (ybuf); VectorE max8 -> candidate buffer.
  - Per 16384-column window: top-8 via max8 over the window's candidates,
    positions via max_index over ybuf (window-local needles are always
    present, so matches are exact f32).
  - 4 windows x 8 = 32 candidates/query; the 5th-largest value is the
    selection threshold; weights 1/(d+eps) masked+normalized on-chip.
  - aux rows gathered per query via indirect DMA; weighted sum on GpSimd.
"""

import numpy as np

from concourse import bass, mybir
from concourse.tile import TileContext
from concourse.bass_utils import run_bass_kernel_spmd

F32 = mybir.dt.float32
U32 = mybir.dt.uint32
I32 = mybir.dt.int32

N_CORES = 8
NQ = 4096
NDB = 65536
D = 32
DAUG = 33
K = 5
EPS = 1e-6

NQ_CORE = NQ // N_CORES          # 512
CHUNK = 512                      # db columns per matmul / max8
WINDOW = 16384                   # db columns per max_index window
CH_PER_WIN = WINDOW // CHUNK     # 32


def build_nc(nq_core=NQ_CORE, ndb=NDB):
    n_qt = nq_core // 128
    n_win = ndb // WINDOW if ndb >= WINDOW else 1
    win = min(WINDOW, ndb)
    ch_per_win = win // CHUNK
    ncand = 8 * n_win

    nc = bass.Bass()
    qT = nc.declare_dram_parameter("qT_aug", [DAUG, nq_core], F32, isOutput=False)
    qsq = nc.declare_dram_parameter("qsq", [nq_core, 1], F32, isOutput=False)
    dbT = nc.declare_dram_parameter("dbT_aug", [DAUG, ndb], F32, isOutput=False)
    aux = nc.declare_dram_parameter("aux", [ndb, D], F32, isOutput=False)
    out = nc.declare_dram_parameter("out", [nq_core, D], F32, isOutput=True)

    with TileContext(nc) as tc:
        with (
            tc.tile_pool(name="ybuf", bufs=1) as ybufp,
            tc.tile_pool(name="db", bufs=3) as dbp,
            tc.tile_pool(name="ps", bufs=6, space="PSUM") as psp,
            tc.tile_pool(name="sm", bufs=2) as sp,
            tc.tile_pool(name="acc", bufs=2) as accp,
        ):
            for t in range(n_qt):
                qt = sp.tile([DAUG, 128], F32, tag="qt")
                nc.sync.dma_start(out=qt[:], in_=qT[:, t * 128:(t + 1) * 128])
                qs = sp.tile([128, 1], F32, tag="qs")
                nc.sync.dma_start(out=qs[:], in_=qsq[t * 128:(t + 1) * 128, :])

                candv = sp.tile([128, ncand], F32, tag="candv")
                candj = sp.tile([128, ncand], F32, tag="candj")

                for w in range(n_win):
                    ybuf = ybufp.tile([128, win], F32)
                    candb = sp.tile([128, 8 * ch_per_win], F32, tag="candb")
                    for c in range(ch_per_win):
                        off = w * win + c * CHUNK
                        rhs = dbp.tile([DAUG, CHUNK], F32)
                        nc.sync.dma_start(out=rhs[:], in_=dbT[:, off:off + CHUNK])
                        ps = psp.tile([128, CHUNK], F32)
                        nc.tensor.matmul(out=ps[:], lhsT=qt[:], rhs=rhs[:],
                                         start=True, stop=True)
                        nc.scalar.copy(out=ybuf[:, c * CHUNK:(c + 1) * CHUNK],
                                       in_=ps[:])
                        nc.vector.max(out=candb[:, c * 8:(c + 1) * 8], in_=ps[:])
                    # window endgame: top-8 values + their positions in ybuf
                    w8 = sp.tile([128, 8], F32, tag="w8")
                    nc.vector.max(out=w8[:], in_=candb[:])
                    pos = sp.tile([128, 8], U32, tag="pos")
                    nc.vector.max_index(out=pos[:], in_max=w8[:], in_values=ybuf[:])
                    posf = sp.tile([128, 8], F32, tag="posf")
                    nc.vector.tensor_copy(posf[:], pos[:])
                    # global j = pos + w*win ; stash value+index
                    nc.vector.tensor_copy(candv[:, w * 8:(w + 1) * 8], w8[:])
                    nc.vector.tensor_scalar_add(candj[:, w * 8:(w + 1) * 8],
                                                posf[:], float(w * win))

                # merge: threshold = 5th largest of all candidates
                t8 = sp.tile([128, 8], F32, tag="t8")
                nc.vector.max(out=t8[:], in_=candv[:])
                mask = sp.tile([128, ncand], F32, tag="mask")
                nc.vector.tensor_scalar(mask[:], candv[:], t8[:, 4:5], None,
                                        op0=mybir.AluOpType.is_ge)
                # d^2 = qsq - 2*negS, clamped at 0
                dsq = sp.tile([128, ncand], F32, tag="dsq")
                nc.vector.tensor_scalar(dsq[:], candv[:], -2.0, qs[:, 0:1],
                                        op0=mybir.AluOpType.mult,
                                        op1=mybir.AluOpType.add)
                nc.vector.tensor_scalar_max(dsq[:], dsq[:], 0.0)
                dist = sp.tile([128, ncand], F32, tag="dist")
                nc.scalar.sqrt(out=dist[:], in_=dsq[:])
                nc.vector.tensor_scalar_add(dist[:], dist[:], EPS)
                rec = sp.tile([128, ncand], F32, tag="rec")
                nc.vector.reciprocal(out=rec[:], in_=dist[:])
                wgt = sp.tile([128, ncand], F32, tag="wgt")
                nc.vector.tensor_tensor(out=wgt[:], in0=rec[:], in1=mask[:],
                                        op=mybir.AluOpType.mult)
                wsum = sp.tile([128, 1], F32, tag="wsum")
                nc.vector.tensor_reduce(out=wsum[:], in_=wgt[:],
                                        axis=mybir.AxisListType.X,
                                        op=mybir.AluOpType.add)
                winv = sp.tile([128, 1], F32, tag="winv")
                nc.vector.reciprocal(out=winv[:], in_=wsum[:])

                # gather aux rows per candidate (per-partition indirect DMA)
                ji = sp.tile([128, ncand], I32, tag="ji")
                nc.vector.tensor_copy(ji[:], candj[:])
                ga = accp.tile([128, ncand, D], F32, tag="ga")
                for i in range(ncand):
                    nc.gpsimd.indirect_dma_start(
                        out=ga[:, i, :],
                        out_offset=None,
                        in_=aux[:],
                        in_offset=bass.IndirectOffsetOnAxis(ap=ji[:, i:i + 1],
                                                            axis=0),
                    )
                # weighted sum (DVE; AP-scalar tensor_scalar is DVE-only)
                acc = accp.tile([128, D], F32, tag="acc0")
                nc.vector.tensor_scalar(acc[:], ga[:, 0, :], wgt[:, 0:1], None,
                                        op0=mybir.AluOpType.mult)
                for i in range(1, ncand):
                    nxt = accp.tile([128, D], F32, tag=f"acc{i % 2}")
                    nc.vector.scalar_tensor_tensor(out=nxt[:], in0=ga[:, i, :],
                                                   scalar=wgt[:, i:i + 1],
                                                   in1=acc[:],
                                                   op0=mybir.AluOpType.mult,
                                                   op1=mybir.AluOpType.add)
                    acc = nxt
                outt = sp.tile([128, D], F32, tag="outt")
                nc.vector.tensor_scalar(outt[:], acc[:], winv[:, 0:1], None,
                                        op0=mybir.AluOpType.mult)
                nc.sync.dma_start(out=out[t * 128:(t + 1) * 128, :], in_=outt[:])

    split_multi_waits(nc)
    return nc


def split_multi_waits(nc):
    """The walrus build in this container supports a single sync-wait per
    instruction; Tile's tail drain carries one wait per live proc.  Split
    any multi-wait instruction into single-wait NoOps ahead of it."""
    for f in nc.m.functions:
        for blk in f.blocks:
            newinsts = []
            for ins in blk.instructions:
                si = ins.sync_info
                if si is not None and si.on_wait and len(si.on_wait) > 1:
                    waits = list(si.on_wait)
                    for k, w in enumerate(waits[:-1]):
                        nop = mybir.InstNoOp(name=f"{ins.name}-ws{k}", ins=[],
                                             outs=[])
                        nop.engine = ins.engine
                        nop.sync_info = mybir.SyncInfo(on_wait=[w], on_update=[])
                        newinsts.append(nop)
                    ins.sync_info = mybir.SyncInfo(on_wait=[waits[-1]],
                                                   on_update=list(si.on_update))
                newinsts.append(ins)
            blk.instructions = newinsts


def make_in_maps(embedding_features, db_embedding, auxiliary_features):
    q = np.ascontiguousarray(np.asarray(embedding_features, dtype=np.float32))
    db = np.ascontiguousarray(np.asarray(db_embedding, dtype=np.float32))
    aux = np.ascontiguousarray(np.asarray(auxiliary_features, dtype=np.float32))
    ndb = db.shape[0]
    nq_core = q.shape[0] // N_CORES
    dbT_aug = np.ascontiguousarray(
        np.concatenate([db.T, (-0.5 * (db * db).sum(1))[None, :]], axis=0)
    ).astype(np.float32)
    in_maps = []
    for c in range(N_CORES):
        qs = q[c * nq_core:(c + 1) * nq_core]
        qT_aug = np.ascontiguousarray(
            np.concatenate([qs.T, np.ones((1, nq_core), np.float32)], axis=0)
        ).astype(np.float32)
        qsq = np.ascontiguousarray((qs * qs).sum(1).reshape(nq_core, 1)
                                   ).astype(np.float32)
        in_maps.append({"qT_aug": qT_aug, "qsq": qsq, "dbT_aug": dbT_aug,
                        "aux": aux})
    return in_maps


_NC_CACHE = {}


def get_nc(nq_core=NQ_CORE, ndb=NDB):
    key = (nq_core, ndb)
    if key not in _NC_CACHE:
        _NC_CACHE[key] = build_nc(nq_core, ndb)
    return _NC_CACHE[key]


def kernel(embedding_features, db_embedding, auxiliary_features):
    in_maps = make_in_maps(embedding_features, db_embedding, auxiliary_features)
    nc = get_nc()
    res = run_bass_kernel_spmd(nc, in_maps, list(range(N_CORES)))
    return np.concatenate([res.results[c]["out"] for c in range(N_CORES)],
                          axis=0).astype(np.float32)


# revision 7
# speedup vs baseline: 1.0343x; 1.0343x over previous
"""Trainium2 Bass kernel for nn_BaseEmbedder (retrieval_knn).

For each of 4096 query embeddings: find the 5 nearest of 65536 db embeddings
(Euclidean) and produce the inverse-distance-weighted sum of their auxiliary
features.  SPMD on 8 NeuronCores: queries sharded 512/core, db+aux replicated.

Per core (512 queries = 4 q-tiles of 128 partitions):
  - Scan phase (bf16): negS[q,j] = q.x_j - 0.5|x_j|^2 via a K=34 augmented
    bf16 matmul (rows 32/33 carry the -0.5|x|^2 bias split hi/lo for
    precision).  TensorE -> PSUM f32; ScalarE copies PSUM -> SBUF f32 ybuf;
    VectorE max8 over 8192-wide ybuf slices -> per-slice top-8 candidates.
  - Position recovery: per 16384-wide window, window top-8 via max8 over the
    slice candidates, then max_index over ybuf (needles are window-local so
    matches are exact f32).  4 windows x 8 = 32 candidates/query.
  - Exact refinement (f32): gather the 32 candidate db rows per query via
    indirect DMA, recompute exact f32 distances on-chip, select top-5 by
    threshold (5th-largest of 2*q.x - |x|^2), weights 1/(d+eps) normalized.
  - aux rows gathered per query via indirect DMA; weighted sum + output DMA.

The bf16 scan only nominates candidates; all selection/weight math is exact
f32, so the result matches the f32 reference to ~1e-6.
"""

import numpy as np
import ml_dtypes

from concourse import bass, mybir
from concourse.tile import TileContext
from concourse.bass_utils import run_bass_kernel_spmd

F32 = mybir.dt.float32
BF16 = mybir.dt.bfloat16
U32 = mybir.dt.uint32
I32 = mybir.dt.int32

N_CORES = 8
NQ = 4096
NDB = 65536
D = 32
DAUG = 34   # 32 dims + bias row + bias-residual row (bf16 split)
K = 5
EPS = 1e-6

NQ_CORE = NQ // N_CORES          # 512
CHUNK = 512                      # db columns per matmul (one PSUM bank)
SUPER = 2048                     # db columns per PSUM tile / ACT copy
BIG = 8192                       # db columns per max8 scan slice (SBUF)
WINDOW = 16384                   # db columns per max_index window


def build_nc(nq_core=NQ_CORE, ndb=NDB):
    n_qt = nq_core // 128
    win = min(WINDOW, ndb)
    n_win = ndb // win
    big = min(BIG, win)
    ncand = 8 * n_win

    nc = bass.Bass()
    qT = nc.declare_dram_parameter("qT_aug", [DAUG, nq_core], BF16, isOutput=False)
    qf = nc.declare_dram_parameter("qf", [nq_core, D], F32, isOutput=False)
    qsq = nc.declare_dram_parameter("qsq", [nq_core, 1], F32, isOutput=False)
    dbT = nc.declare_dram_parameter("dbT_aug", [DAUG, ndb], BF16, isOutput=False)
    dbr = nc.declare_dram_parameter("db_rows", [ndb, D], F32, isOutput=False)
    aux = nc.declare_dram_parameter("aux", [ndb, D], F32, isOutput=False)
    out = nc.declare_dram_parameter("out", [nq_core, D], F32, isOutput=True)

    with TileContext(nc) as tc:
        with (
            tc.tile_pool(name="ybuf", bufs=1) as ybufp,
            tc.tile_pool(name="db", bufs=3) as dbp,
            tc.tile_pool(name="ps", bufs=2, space="PSUM") as psp,
            tc.tile_pool(name="sm", bufs=2) as sp,
            tc.tile_pool(name="g", bufs=2) as gp,
        ):
            for t in range(n_qt):
                qt = sp.tile([DAUG, 128], BF16, tag="qt")
                nc.sync.dma_start(out=qt[:], in_=qT[:, t * 128:(t + 1) * 128])
                qs = sp.tile([128, 1], F32, tag="qs")
                nc.sync.dma_start(out=qs[:], in_=qsq[t * 128:(t + 1) * 128, :])
                qft = sp.tile([128, D], F32, tag="qft")
                nc.sync.dma_start(out=qft[:], in_=qf[t * 128:(t + 1) * 128, :])

                candv = sp.tile([128, ncand], F32, tag="candv")
                candj = sp.tile([128, ncand], F32, tag="candj")

                for w in range(n_win):
                    ybuf = ybufp.tile([128, win], F32)
                    nbig = win // big
                    candb = sp.tile([128, 8 * nbig], F32, tag="candb")
                    for c in range(win // SUPER):
                        off = w * win + c * SUPER
                        rhs = dbp.tile([DAUG, SUPER], BF16)
                        nc.sync.dma_start(out=rhs[:], in_=dbT[:, off:off + SUPER])
                        ps = psp.tile([128, SUPER], F32)
                        for m in range(SUPER // CHUNK):
                            nc.tensor.matmul(
                                out=ps[:, m * CHUNK:(m + 1) * CHUNK],
                                lhsT=qt[:],
                                rhs=rhs[:, m * CHUNK:(m + 1) * CHUNK],
                                start=True, stop=True)
                        nc.scalar.copy(out=ybuf[:, c * SUPER:(c + 1) * SUPER],
                                       in_=ps[:])
                    for c in range(nbig):
                        nc.vector.max(out=candb[:, c * 8:(c + 1) * 8],
                                      in_=ybuf[:, c * big:(c + 1) * big])
                    # window top-8 values + their positions in ybuf
                    w8 = candv[:, w * 8:(w + 1) * 8]
                    nc.vector.max(out=w8, in_=candb[:])
                    pos = sp.tile([128, 8], U32, tag="pos")
                    nc.vector.max_index(out=pos[:], in_max=w8, in_values=ybuf[:])
                    # global j = pos + w*win  (u32 -> f32 exact)
                    nc.vector.tensor_scalar_add(candj[:, w * 8:(w + 1) * 8],
                                                pos[:], float(w * win))

                # ---- exact f32 refinement over the 32 candidates ----
                ji = sp.tile([128, ncand], I32, tag="ji")
                nc.vector.tensor_copy(ji[:], candj[:])
                gx = gp.tile([128, ncand, D], F32, tag="gx")
                ga = gp.tile([128, ncand, D], F32, tag="ga")
                for i in range(ncand):
                    nc.gpsimd.indirect_dma_start(
                        out=gx[:, i, :], out_offset=None, in_=dbr[:],
                        in_offset=bass.IndirectOffsetOnAxis(ap=ji[:, i:i + 1],
                                                            axis=0))
                    nc.gpsimd.indirect_dma_start(
                        out=ga[:, i, :], out_offset=None, in_=aux[:],
                        in_offset=bass.IndirectOffsetOnAxis(ap=ji[:, i:i + 1],
                                                            axis=0))
                # dots[q,c] = q . x_c ; xsq[q,c] = |x_c|^2
                pr = gp.tile([128, ncand, D], F32, tag="pr")
                nc.vector.tensor_tensor(
                    out=pr[:], in0=gx[:],
                    in1=qft[:].unsqueeze(1).to_broadcast([128, ncand, D]),
                    op=mybir.AluOpType.mult)
                dots = sp.tile([128, ncand], F32, tag="dots")
                nc.vector.tensor_reduce(out=dots[:], in_=pr[:],
                                        axis=mybir.AxisListType.X,
                                        op=mybir.AluOpType.add)
                sq = gp.tile([128, ncand, D], F32, tag="sq")
                nc.scalar.square(out=sq[:], in_=gx[:])
                xsq = sp.tile([128, ncand], F32, tag="xsq")
                nc.vector.tensor_reduce(out=xsq[:], in_=sq[:],
                                        axis=mybir.AxisListType.X,
                                        op=mybir.AluOpType.add)
                # neg2 = 2*dots - xsq  (=> dsq = qsq - neg2); top-5 by neg2
                neg2 = sp.tile([128, ncand], F32, tag="neg2")
                nc.vector.scalar_tensor_tensor(out=neg2[:], in0=dots[:],
                                               scalar=2.0, in1=xsq[:],
                                               op0=mybir.AluOpType.mult,
                                               op1=mybir.AluOpType.subtract)
                t8 = sp.tile([128, 8], F32, tag="t8")
                nc.vector.max(out=t8[:], in_=neg2[:])
                mask = sp.tile([128, ncand], F32, tag="mask")
                nc.vector.tensor_scalar(mask[:], neg2[:], t8[:, 4:5], None,
                                        op0=mybir.AluOpType.is_ge)
                dsq = sp.tile([128, ncand], F32, tag="dsq")
                nc.vector.tensor_scalar(dsq[:], neg2[:], -1.0, qs[:, 0:1],
                                        op0=mybir.AluOpType.mult,
                                        op1=mybir.AluOpType.add)
                nc.vector.tensor_scalar_max(dsq[:], dsq[:], 0.0)
                dist = sp.tile([128, ncand], F32, tag="dist")
                nc.scalar.sqrt(out=dist[:], in_=dsq[:])
                nc.vector.tensor_scalar_add(dist[:], dist[:], EPS)
                rec = sp.tile([128, ncand], F32, tag="rec")
                nc.vector.reciprocal(out=rec[:], in_=dist[:])
                wgt = sp.tile([128, ncand], F32, tag="wgt")
                nc.vector.tensor_tensor(out=wgt[:], in0=rec[:], in1=mask[:],
                                        op=mybir.AluOpType.mult)
                wsum = sp.tile([128, 1], F32, tag="wsum")
                nc.vector.tensor_reduce(out=wsum[:], in_=wgt[:],
                                        axis=mybir.AxisListType.X,
                                        op=mybir.AluOpType.add)
                winv = sp.tile([128, 1], F32, tag="winv")
                nc.vector.reciprocal(out=winv[:], in_=wsum[:])

                # weighted sum of gathered aux rows
                prod = gp.tile([128, ncand, D], F32, tag="prod")
                nc.vector.tensor_tensor(
                    out=prod[:], in0=ga[:],
                    in1=wgt[:].unsqueeze(-1).to_broadcast([128, ncand, D]),
                    op=mybir.AluOpType.mult)
                acc = sp.tile([128, D], F32, tag="accr")
                nc.vector.tensor_reduce(
                    out=acc[:], in_=prod[:].rearrange("p i a -> p a i"),
                    axis=mybir.AxisListType.X, op=mybir.AluOpType.add)
                outt = sp.tile([128, D], F32, tag="outt")
                nc.vector.tensor_scalar(outt[:], acc[:], winv[:, 0:1], None,
                                        op0=mybir.AluOpType.mult)
                nc.sync.dma_start(out=out[t * 128:(t + 1) * 128, :], in_=outt[:])

    split_multi_waits(nc)
    return nc


def split_multi_waits(nc):
    """The walrus build in this container supports a single sync-wait per
    instruction; Tile's tail drain carries one wait per live proc.  Split
    any multi-wait instruction into single-wait NoOps ahead of it."""
    for f in nc.m.functions:
        for blk in f.blocks:
            newinsts = []
            for ins in blk.instructions:
                si = ins.sync_info
                if si is not None and si.on_wait and len(si.on_wait) > 1:
                    waits = list(si.on_wait)
                    for k, w in enumerate(waits[:-1]):
                        nop = mybir.InstNoOp(name=f"{ins.name}-ws{k}", ins=[],
                                             outs=[])
                        nop.engine = ins.engine
                        nop.sync_info = mybir.SyncInfo(on_wait=[w], on_update=[])
                        newinsts.append(nop)
                    ins.sync_info = mybir.SyncInfo(on_wait=[waits[-1]],
                                                   on_update=list(si.on_update))
                newinsts.append(ins)
            blk.instructions = newinsts


def make_in_maps(embedding_features, db_embedding, auxiliary_features):
    q = np.ascontiguousarray(np.asarray(embedding_features, dtype=np.float32))
    db = np.ascontiguousarray(np.asarray(db_embedding, dtype=np.float32))
    aux = np.ascontiguousarray(np.asarray(auxiliary_features, dtype=np.float32))
    nq_core = q.shape[0] // N_CORES
    bf = ml_dtypes.bfloat16
    bias = -0.5 * (db * db).sum(1)                      # exact f32
    b_hi = bias.astype(bf).astype(np.float32)
    b_lo = (bias - b_hi).astype(bf)
    dbT_aug = np.ascontiguousarray(np.concatenate(
        [db.T.astype(bf), b_hi.astype(bf)[None, :], b_lo[None, :]], axis=0,
        dtype=bf))
    in_maps = []
    for c in range(N_CORES):
        qs = q[c * nq_core:(c + 1) * nq_core]
        qT_aug = np.ascontiguousarray(np.concatenate(
            [qs.T.astype(bf), np.ones((2, nq_core), bf)], axis=0, dtype=bf))
        qsq = np.ascontiguousarray((qs * qs).sum(1).reshape(nq_core, 1)
                                   ).astype(np.float32)
        in_maps.append({"qT_aug": qT_aug, "qf": qs, "qsq": qsq,
                        "dbT_aug": dbT_aug, "db_rows": db, "aux": aux})
    return in_maps


_NC_CACHE = {}


def get_nc(nq_core=NQ_CORE, ndb=NDB):
    key = (nq_core, ndb)
    if key not in _NC_CACHE:
        _NC_CACHE[key] = build_nc(nq_core, ndb)
    return _NC_CACHE[key]


def kernel(embedding_features, db_embedding, auxiliary_features):
    in_maps = make_in_maps(embedding_features, db_embedding, auxiliary_features)
    nc = get_nc()
    res = run_bass_kernel_spmd(nc, in_maps, list(range(N_CORES)))
    return np.concatenate([res.results[c]["out"] for c in range(N_CORES)],
                          axis=0).astype(np.float32)


# revision 9
# speedup vs baseline: 1.6288x; 1.5748x over previous
"""Trainium2 Bass kernel for nn_BaseEmbedder (retrieval_knn).

For each of 4096 query embeddings: find the 5 nearest of 65536 db embeddings
(Euclidean) and produce the inverse-distance-weighted sum of their auxiliary
features.  SPMD on 8 NeuronCores: queries sharded 512/core, db+aux replicated.

Per core (512 queries = 4 q-tiles of 128 partitions):
  - Scan phase (bf16): negS[q,j] = q.x_j - 0.5|x_j|^2 via a K=34 augmented
    bf16 matmul (rows 32/33 carry the -0.5|x|^2 bias split hi/lo for
    precision).  TensorE -> PSUM f32; ScalarE copies PSUM -> SBUF f32 ybuf;
    VectorE max8 over 8192-wide ybuf slices -> per-slice top-8 candidates.
  - Position recovery: per 16384-wide window, window top-8 via max8 over the
    slice candidates, then max_index over ybuf (needles are window-local so
    matches are exact f32).  4 windows x 8 = 32 candidates/query.
  - Exact refinement (f32): gather the 32 candidate db rows per query via
    indirect DMA, recompute exact f32 distances on-chip, select top-5 by
    threshold (5th-largest of 2*q.x - |x|^2), weights 1/(d+eps) normalized.
  - aux rows gathered per query via indirect DMA; weighted sum + output DMA.

The bf16 scan only nominates candidates; all selection/weight math is exact
f32, so the result matches the f32 reference to ~1e-6.
"""

import numpy as np
import ml_dtypes

from concourse import bass, mybir
from concourse.tile import TileContext
from concourse.bass_utils import run_bass_kernel_spmd

F32 = mybir.dt.float32
BF16 = mybir.dt.bfloat16
U32 = mybir.dt.uint32
I32 = mybir.dt.int32

N_CORES = 8
NQ = 4096
NDB = 65536
D = 32
DAUG = 34   # 32 dims + bias row + bias-residual row (bf16 split)
K = 5
EPS = 1e-6

NQ_CORE = NQ // N_CORES          # 512
CHUNK = 512                      # db columns per matmul (one PSUM bank)
SUPER = 2048                     # db columns per PSUM tile / ACT copy
WINDOW = 16384                   # db columns per max8 + max_index window
RG_B = 64                        # partition base of the second PE row-group


def build_nc(nq_core=NQ_CORE, ndb=NDB):
    n_qt = nq_core // 128
    win = min(WINDOW, ndb)
    n_win = ndb // win
    ncand = 8 * n_win

    nc = bass.Bass()
    qT = nc.declare_dram_parameter("qT_aug", [DAUG, nq_core], BF16, isOutput=False)
    qf = nc.declare_dram_parameter("qf", [nq_core, D], F32, isOutput=False)
    qsq = nc.declare_dram_parameter("qsq", [nq_core, 1], F32, isOutput=False)
    dbT = nc.declare_dram_parameter("dbT_aug", [DAUG, ndb], BF16, isOutput=False)
    dbaux = nc.declare_dram_parameter("dbaux", [ndb, 2 * D], F32, isOutput=False)
    out = nc.declare_dram_parameter("out", [nq_core, D], F32, isOutput=True)

    with TileContext(nc) as tc:
        with (
            tc.tile_pool(name="ybuf", bufs=2) as ybufp,
            tc.tile_pool(name="db", bufs=3) as dbp,
            tc.tile_pool(name="psA", bufs=1, space="PSUM") as pspA,
            tc.tile_pool(name="psB", bufs=1, space="PSUM") as pspB,
            tc.tile_pool(name="sm", bufs=2) as sp,
            tc.tile_pool(name="g", bufs=1) as gp,
        ):
            for t in range(n_qt):
                # queries live twice: row group 0 and row group RG_B, so two
                # matmul streams run concurrently on the PE array
                qt = sp.tile([128, 128], BF16, tag="qt")
                nc.sync.dma_start(out=qt[0:DAUG, :],
                                  in_=qT[:, t * 128:(t + 1) * 128])
                nc.sync.dma_start(out=qt[RG_B:RG_B + DAUG, :],
                                  in_=qT[:, t * 128:(t + 1) * 128])
                qs = sp.tile([128, 1], F32, tag="qs")
                nc.sync.dma_start(out=qs[:], in_=qsq[t * 128:(t + 1) * 128, :])
                qft = sp.tile([128, D], F32, tag="qft")
                nc.sync.dma_start(out=qft[:], in_=qf[t * 128:(t + 1) * 128, :])

                candv = sp.tile([128, ncand], F32, tag="candv")
                candj = sp.tile([128, ncand], F32, tag="candj")

                for w in range(n_win):
                    ybuf = ybufp.tile([128, win], F32)
                    for p in range(win // (2 * SUPER)):
                        offa = w * win + (2 * p) * SUPER
                        rhs = dbp.tile([128, SUPER], BF16)
                        nc.sync.dma_start(out=rhs[0:DAUG, :],
                                          in_=dbT[:, offa:offa + SUPER])
                        nc.sync.dma_start(
                            out=rhs[RG_B:RG_B + DAUG, :],
                            in_=dbT[:, offa + SUPER:offa + 2 * SUPER])
                        psA = pspA.tile([128, SUPER], F32, tag="psA")
                        psB = pspB.tile([128, SUPER], F32, tag="psB")
                        for m in range(SUPER // CHUNK):
                            sl = slice(m * CHUNK, (m + 1) * CHUNK)
                            nc.tensor.matmul(out=psA[:, sl],
                                             lhsT=qt[0:DAUG, :],
                                             rhs=rhs[0:DAUG, sl],
                                             start=True, stop=True,
                                             tile_position=(0, 0))
                            nc.tensor.matmul(out=psB[:, sl],
                                             lhsT=qt[RG_B:RG_B + DAUG, :],
                                             rhs=rhs[RG_B:RG_B + DAUG, sl],
                                             start=True, stop=True,
                                             tile_position=(RG_B, 0))
                        ya = (2 * p) * SUPER
                        nc.scalar.copy(out=ybuf[:, ya:ya + SUPER], in_=psA[:])
                        nc.scalar.copy(out=ybuf[:, ya + SUPER:ya + 2 * SUPER],
                                       in_=psB[:])
                    # window top-8 values + their positions in ybuf
                    w8 = candv[:, w * 8:(w + 1) * 8]
                    nc.vector.max(out=w8, in_=ybuf[:])
                    pos = sp.tile([128, 8], U32, tag="pos")
                    nc.vector.max_index(out=pos[:], in_max=w8, in_values=ybuf[:])
                    # global j = pos + w*win  (u32 -> f32 exact)
                    nc.vector.tensor_scalar_add(candj[:, w * 8:(w + 1) * 8],
                                                pos[:], float(w * win))

                # ---- exact f32 refinement over the 32 candidates ----
                ji = sp.tile([128, ncand], I32, tag="ji")
                nc.vector.tensor_copy(ji[:], candj[:])
                gxa = gp.tile([128, ncand, 2 * D], F32, tag="gxa")
                for i in range(ncand):
                    nc.gpsimd.indirect_dma_start(
                        out=gxa[:, i, :], out_offset=None, in_=dbaux[:],
                        in_offset=bass.IndirectOffsetOnAxis(ap=ji[:, i:i + 1],
                                                            axis=0))
                gx = gxa[:, :, 0:D]
                ga = gxa[:, :, D:2 * D]
                # dots[q,c] = q . x_c ; xsq[q,c] = |x_c|^2
                pr = gp.tile([128, ncand, D], F32, tag="pr")
                nc.vector.tensor_tensor(
                    out=pr[:], in0=gx,
                    in1=qft[:].unsqueeze(1).to_broadcast([128, ncand, D]),
                    op=mybir.AluOpType.mult)
                dots = sp.tile([128, ncand], F32, tag="dots")
                nc.vector.tensor_reduce(out=dots[:], in_=pr[:],
                                        axis=mybir.AxisListType.X,
                                        op=mybir.AluOpType.add)
                sq = gp.tile([128, ncand, D], F32, tag="sq")
                nc.scalar.square(out=sq[:], in_=gx)
                xsq = sp.tile([128, ncand], F32, tag="xsq")
                nc.vector.tensor_reduce(out=xsq[:], in_=sq[:],
                                        axis=mybir.AxisListType.X,
                                        op=mybir.AluOpType.add)
                # neg2 = 2*dots - xsq  (=> dsq = qsq - neg2); top-5 by neg2
                neg2 = sp.tile([128, ncand], F32, tag="neg2")
                nc.vector.scalar_tensor_tensor(out=neg2[:], in0=dots[:],
                                               scalar=2.0, in1=xsq[:],
                                               op0=mybir.AluOpType.mult,
                                               op1=mybir.AluOpType.subtract)
                t8 = sp.tile([128, 8], F32, tag="t8")
                nc.vector.max(out=t8[:], in_=neg2[:])
                mask = sp.tile([128, ncand], F32, tag="mask")
                nc.vector.tensor_scalar(mask[:], neg2[:], t8[:, 4:5], None,
                                        op0=mybir.AluOpType.is_ge)
                dsq = sp.tile([128, ncand], F32, tag="dsq")
                nc.vector.tensor_scalar(dsq[:], neg2[:], -1.0, qs[:, 0:1],
                                        op0=mybir.AluOpType.mult,
                                        op1=mybir.AluOpType.add)
                nc.vector.tensor_scalar_max(dsq[:], dsq[:], 0.0)
                dist = sp.tile([128, ncand], F32, tag="dist")
                nc.scalar.sqrt(out=dist[:], in_=dsq[:])
                nc.vector.tensor_scalar_add(dist[:], dist[:], EPS)
                rec = sp.tile([128, ncand], F32, tag="rec")
                nc.vector.reciprocal(out=rec[:], in_=dist[:])
                wgt = sp.tile([128, ncand], F32, tag="wgt")
                nc.vector.tensor_tensor(out=wgt[:], in0=rec[:], in1=mask[:],
                                        op=mybir.AluOpType.mult)
                wsum = sp.tile([128, 1], F32, tag="wsum")
                nc.vector.tensor_reduce(out=wsum[:], in_=wgt[:],
                                        axis=mybir.AxisListType.X,
                                        op=mybir.AluOpType.add)
                winv = sp.tile([128, 1], F32, tag="winv")
                nc.vector.reciprocal(out=winv[:], in_=wsum[:])

                # weighted sum of gathered aux rows
                prod = gp.tile([128, ncand, D], F32, tag="prod")
                nc.vector.tensor_tensor(
                    out=prod[:], in0=ga,
                    in1=wgt[:].unsqueeze(-1).to_broadcast([128, ncand, D]),
                    op=mybir.AluOpType.mult)
                acc = sp.tile([128, D], F32, tag="accr")
                nc.vector.tensor_reduce(
                    out=acc[:], in_=prod[:].rearrange("p i a -> p a i"),
                    axis=mybir.AxisListType.X, op=mybir.AluOpType.add)
                outt = sp.tile([128, D], F32, tag="outt")
                nc.vector.tensor_scalar(outt[:], acc[:], winv[:, 0:1], None,
                                        op0=mybir.AluOpType.mult)
                nc.sync.dma_start(out=out[t * 128:(t + 1) * 128, :], in_=outt[:])

    split_multi_waits(nc)
    return nc


def split_multi_waits(nc):
    """The walrus build in this container supports a single sync-wait per
    instruction; Tile's tail drain carries one wait per live proc.  Split
    any multi-wait instruction into single-wait NoOps ahead of it."""
    for f in nc.m.functions:
        for blk in f.blocks:
            newinsts = []
            for ins in blk.instructions:
                si = ins.sync_info
                if si is not None and si.on_wait and len(si.on_wait) > 1:
                    waits = list(si.on_wait)
                    for k, w in enumerate(waits[:-1]):
                        nop = mybir.InstNoOp(name=f"{ins.name}-ws{k}", ins=[],
                                             outs=[])
                        nop.engine = ins.engine
                        nop.sync_info = mybir.SyncInfo(on_wait=[w], on_update=[])
                        newinsts.append(nop)
                    ins.sync_info = mybir.SyncInfo(on_wait=[waits[-1]],
                                                   on_update=list(si.on_update))
                newinsts.append(ins)
            blk.instructions = newinsts


def make_in_maps(embedding_features, db_embedding, auxiliary_features):
    q = np.ascontiguousarray(np.asarray(embedding_features, dtype=np.float32))
    db = np.ascontiguousarray(np.asarray(db_embedding, dtype=np.float32))
    aux = np.ascontiguousarray(np.asarray(auxiliary_features, dtype=np.float32))
    nq_core = q.shape[0] // N_CORES
    bf = ml_dtypes.bfloat16
    bias = -0.5 * (db * db).sum(1)                      # exact f32
    b_hi = bias.astype(bf).astype(np.float32)
    b_lo = (bias - b_hi).astype(bf)
    dbT_aug = np.ascontiguousarray(np.concatenate(
        [db.T.astype(bf), b_hi.astype(bf)[None, :], b_lo[None, :]], axis=0,
        dtype=bf))
    dbaux = np.ascontiguousarray(np.concatenate([db, aux], axis=1))
    in_maps = []
    for c in range(N_CORES):
        qs = q[c * nq_core:(c + 1) * nq_core]
        qT_aug = np.ascontiguousarray(np.concatenate(
            [qs.T.astype(bf), np.ones((2, nq_core), bf)], axis=0, dtype=bf))
        qsq = np.ascontiguousarray((qs * qs).sum(1).reshape(nq_core, 1)
                                   ).astype(np.float32)
        in_maps.append({"qT_aug": qT_aug, "qf": qs, "qsq": qsq,
                        "dbT_aug": dbT_aug, "dbaux": dbaux})
    return in_maps


_NC_CACHE = {}


def get_nc(nq_core=NQ_CORE, ndb=NDB):
    key = (nq_core, ndb)
    if key not in _NC_CACHE:
        _NC_CACHE[key] = build_nc(nq_core, ndb)
    return _NC_CACHE[key]


def kernel(embedding_features, db_embedding, auxiliary_features):
    in_maps = make_in_maps(embedding_features, db_embedding, auxiliary_features)
    nc = get_nc()
    res = run_bass_kernel_spmd(nc, in_maps, list(range(N_CORES)))
    return np.concatenate([res.results[c]["out"] for c in range(N_CORES)],
                          axis=0).astype(np.float32)


# revision 11
# speedup vs baseline: 1.8031x; 1.1070x over previous
"""Trainium2 Bass kernel for nn_BaseEmbedder (retrieval_knn).

For each of 4096 query embeddings: find the 5 nearest of 65536 db embeddings
(Euclidean) and produce the inverse-distance-weighted sum of their auxiliary
features.  SPMD on 8 NeuronCores: queries sharded 512/core, db+aux replicated.

Per core (512 queries = 4 q-tiles of 128 partitions):
  - Scan (bf16): negS[q,j] = q.x_j - 0.5|x_j|^2 via K=34 augmented bf16
    matmuls (rows 32/33 carry the -0.5|x|^2 bias split hi/lo).  Two matmul
    streams run concurrently on PE row-groups 0 and 64 (even/odd 1024-col
    supers).
  - Pair-fold: DVE tensor_tensor(max) folds each super pair (PSUM A operand,
    SBUF copy of B) into zfold[u] = max(y_even[u], y_odd[u]) - 8192 values
    per 16384-column window.
  - Candidates: per window, max8 over zfold gives the top-8 folded values;
    max_index recovers their fold slots (needles are window-local, exact f32
    match).  Each slot maps to TWO db rows (the fold pair); both are
    gathered, so no parity disambiguation is needed.
  - Exact refinement (f32): a host-prepared paired table
    row[w*8192+p*1024+u] = [x_j1, aux_j1, x_j2, aux_j2, |x_j1|^2, |x_j2|^2]
    is gathered per winning slot via indirect DMA (one row per candidate
    pair per query).  Exact distances for all 64 candidates are recomputed
    on-chip; top-5 by threshold; weights 1/(d+eps) normalized; weighted sum.

The bf16 scan only nominates candidates; all selection/weight math is exact
f32, so the result matches the f32 reference to ~1e-6.
"""

import numpy as np
import ml_dtypes

from concourse import bass, mybir
from concourse.tile import TileContext
from concourse.bass_utils import run_bass_kernel_spmd

F32 = mybir.dt.float32
BF16 = mybir.dt.bfloat16
U32 = mybir.dt.uint32
I32 = mybir.dt.int32

N_CORES = 8
NQ = 4096
NDB = 65536
D = 32
DAUG = 34   # 32 dims + bias row + bias-residual row (bf16 split)
K = 5
EPS = 1e-6

NQ_CORE = NQ // N_CORES          # 512
CHUNK = 512                      # db columns per matmul (one PSUM bank)
SUPER = 1024                     # db columns per PSUM tile / fold operand
WINDOW = 16384                   # db columns per max8 + max_index window
RG_B = 64                        # partition base of the second PE row-group
PV = 132                         # paired-table row: xA,auxA,xB,auxB,xsqA,xsqB,pad


def build_nc(nq_core=NQ_CORE, ndb=NDB):
    n_qt = nq_core // 128
    win = min(WINDOW, ndb)
    n_win = ndb // win
    n_pair = win // (2 * SUPER)          # fold pairs per window (8)
    fold_w = win // 2                    # folded columns per window (8192)
    npair_c = 8 * n_win                  # candidate pairs per query
    ncand = 2 * npair_c                  # candidates per query

    nc = bass.Bass()
    qT = nc.declare_dram_parameter("qT_aug", [DAUG, nq_core], BF16, isOutput=False)
    qf = nc.declare_dram_parameter("qf", [nq_core, D], F32, isOutput=False)
    qsq = nc.declare_dram_parameter("qsq", [nq_core, 1], F32, isOutput=False)
    dbT = nc.declare_dram_parameter("dbT_aug", [DAUG, ndb], BF16, isOutput=False)
    pairt = nc.declare_dram_parameter("pair_table", [ndb // 2, PV], F32,
                                      isOutput=False)
    out = nc.declare_dram_parameter("out", [nq_core, D], F32, isOutput=True)

    with TileContext(nc) as tc:
        with (
            tc.tile_pool(name="zf", bufs=1) as zfp,
            tc.tile_pool(name="db", bufs=3) as dbp,
            tc.tile_pool(name="sbB", bufs=3) as sbp,
            tc.tile_pool(name="psA", bufs=2, space="PSUM") as pspA,
            tc.tile_pool(name="psB", bufs=2, space="PSUM") as pspB,
            tc.tile_pool(name="sm", bufs=2) as sp,
            tc.tile_pool(name="g", bufs=2) as gp,
        ):
            for t in range(n_qt):
                # queries live on row groups 0 and RG_B so two matmul streams
                # run concurrently on the PE array
                qt = sp.tile([128, 128], BF16, tag="qt")
                nc.sync.dma_start(out=qt[0:DAUG, :],
                                  in_=qT[:, t * 128:(t + 1) * 128])
                nc.sync.dma_start(out=qt[RG_B:RG_B + DAUG, :],
                                  in_=qT[:, t * 128:(t + 1) * 128])
                qs = sp.tile([128, 1], F32, tag="qs")
                nc.sync.dma_start(out=qs[:], in_=qsq[t * 128:(t + 1) * 128, :])
                qft = sp.tile([128, D], F32, tag="qft")
                nc.sync.dma_start(out=qft[:], in_=qf[t * 128:(t + 1) * 128, :])

                candv = sp.tile([128, npair_c], F32, tag="candv")
                gxa = gp.tile([128, npair_c, PV], F32, tag="gxa")

                for w in range(n_win):
                    zfold = zfp.tile([128, fold_w], F32)
                    for p in range(n_pair):
                        offa = w * win + (2 * p) * SUPER
                        rhs = dbp.tile([128, SUPER], BF16)
                        nc.sync.dma_start(out=rhs[0:DAUG, :],
                                          in_=dbT[:, offa:offa + SUPER])
                        nc.sync.dma_start(
                            out=rhs[RG_B:RG_B + DAUG, :],
                            in_=dbT[:, offa + SUPER:offa + 2 * SUPER])
                        psA = pspA.tile([128, SUPER], F32, tag="psA")
                        psB = pspB.tile([128, SUPER], F32, tag="psB")
                        for m in range(SUPER // CHUNK):
                            sl = slice(m * CHUNK, (m + 1) * CHUNK)
                            nc.tensor.matmul(out=psA[:, sl],
                                             lhsT=qt[0:DAUG, :],
                                             rhs=rhs[0:DAUG, sl],
                                             start=True, stop=True,
                                             tile_position=(0, 0))
                            nc.tensor.matmul(out=psB[:, sl],
                                             lhsT=qt[RG_B:RG_B + DAUG, :],
                                             rhs=rhs[RG_B:RG_B + DAUG, sl],
                                             start=True, stop=True,
                                             tile_position=(RG_B, 0))
                        sbB = sbp.tile([128, SUPER], F32)
                        nc.scalar.copy(out=sbB[:], in_=psB[:])
                        nc.vector.tensor_tensor(
                            out=zfold[:, p * SUPER:(p + 1) * SUPER],
                            in0=psA[:], in1=sbB[:], op=mybir.AluOpType.max)
                    # window top-8 folded values + their fold slots
                    w8 = candv[:, w * 8:(w + 1) * 8]
                    nc.vector.max(out=w8, in_=zfold[:])
                    pos = sp.tile([128, 8], U32, tag="pos")
                    nc.vector.max_index(out=pos[:], in_max=w8,
                                        in_values=zfold[:])
                    # paired-table row = slot + w*fold_w; gather immediately
                    ji = sp.tile([128, 8], I32, tag="ji")
                    nc.vector.tensor_scalar_add(ji[:], pos[:],
                                                float(w * fold_w))
                    for i in range(8):
                        nc.gpsimd.indirect_dma_start(
                            out=gxa[:, w * 8 + i, :], out_offset=None,
                            in_=pairt[:],
                            in_offset=bass.IndirectOffsetOnAxis(
                                ap=ji[:, i:i + 1], axis=0))

                # ---- exact f32 refinement over the ncand candidates ----
                # gxa row: [xA(32) auxA(32) xB(32) auxB(32) xsqA xsqB pad2]
                base = gxa[:, :, 0:4 * D].rearrange("p c (h v) -> p c h v", h=2)
                gx = base[:, :, :, 0:D]
                ga = base[:, :, :, D:2 * D]
                xsq = gxa[:, :, 4 * D:4 * D + 2]          # [128, npair_c, 2]
                # dots[q, c, h] = q . x
                pr = gp.tile([128, npair_c, 2, D], F32, tag="pr")
                nc.vector.tensor_tensor(
                    out=pr[:], in0=gx,
                    in1=qft[:].unsqueeze(1).unsqueeze(1)
                              .to_broadcast([128, npair_c, 2, D]),
                    op=mybir.AluOpType.mult)
                dots = sp.tile([128, npair_c, 2], F32, tag="dots")
                nc.vector.tensor_reduce(out=dots[:], in_=pr[:],
                                        axis=mybir.AxisListType.X,
                                        op=mybir.AluOpType.add)
                # neg2 = 2*dots - xsq  (dsq = qsq - neg2)
                neg2 = sp.tile([128, ncand], F32, tag="neg2")
                nc.vector.scalar_tensor_tensor(
                    out=neg2[:].rearrange("p (c h) -> p c h", h=2),
                    in0=dots[:], scalar=2.0, in1=xsq,
                    op0=mybir.AluOpType.mult, op1=mybir.AluOpType.subtract)
                t8 = sp.tile([128, 8], F32, tag="t8")
                nc.vector.max(out=t8[:], in_=neg2[:])
                mask = sp.tile([128, ncand], F32, tag="mask")
                nc.vector.tensor_scalar(mask[:], neg2[:], t8[:, 4:5], None,
                                        op0=mybir.AluOpType.is_ge)
                dsq = sp.tile([128, ncand], F32, tag="dsq")
                nc.vector.tensor_scalar(dsq[:], neg2[:], -1.0, qs[:, 0:1],
                                        op0=mybir.AluOpType.mult,
                                        op1=mybir.AluOpType.add)
                nc.vector.tensor_scalar_max(dsq[:], dsq[:], 0.0)
                dist = sp.tile([128, ncand], F32, tag="dist")
                nc.scalar.sqrt(out=dist[:], in_=dsq[:])
                nc.vector.tensor_scalar_add(dist[:], dist[:], EPS)
                rec = sp.tile([128, ncand], F32, tag="rec")
                nc.vector.reciprocal(out=rec[:], in_=dist[:])
                wgt = sp.tile([128, ncand], F32, tag="wgt")
                nc.vector.tensor_tensor(out=wgt[:], in0=rec[:], in1=mask[:],
                                        op=mybir.AluOpType.mult)
                wsum = sp.tile([128, 1], F32, tag="wsum")
                nc.vector.tensor_reduce(out=wsum[:], in_=wgt[:],
                                        axis=mybir.AxisListType.X,
                                        op=mybir.AluOpType.add)
                winv = sp.tile([128, 1], F32, tag="winv")
                nc.vector.reciprocal(out=winv[:], in_=wsum[:])

                # weighted sum of gathered aux rows
                prod = gp.tile([128, npair_c, 2, D], F32, tag="prod")
                nc.vector.tensor_tensor(
                    out=prod[:], in0=ga,
                    in1=wgt[:].rearrange("p (c h) -> p c h", h=2).unsqueeze(-1)
                              .to_broadcast([128, npair_c, 2, D]),
                    op=mybir.AluOpType.mult)
                acc = sp.tile([128, D], F32, tag="accr")
                nc.vector.tensor_reduce(
                    out=acc[:],
                    in_=prod[:].rearrange("p i h a -> p a (i h)"),
                    axis=mybir.AxisListType.X, op=mybir.AluOpType.add)
                outt = sp.tile([128, D], F32, tag="outt")
                nc.vector.tensor_scalar(outt[:], acc[:], winv[:, 0:1], None,
                                        op0=mybir.AluOpType.mult)
                nc.sync.dma_start(out=out[t * 128:(t + 1) * 128, :], in_=outt[:])

    split_multi_waits(nc)
    return nc


def split_multi_waits(nc):
    """The walrus build in this container supports a single sync-wait per
    instruction; Tile's tail drain carries one wait per live proc.  Split
    any multi-wait instruction into single-wait NoOps ahead of it."""
    for f in nc.m.functions:
        for blk in f.blocks:
            newinsts = []
            for ins in blk.instructions:
                si = ins.sync_info
                if si is not None and si.on_wait and len(si.on_wait) > 1:
                    waits = list(si.on_wait)
                    for k, w in enumerate(waits[:-1]):
                        nop = mybir.InstNoOp(name=f"{ins.name}-ws{k}", ins=[],
                                             outs=[])
                        nop.engine = ins.engine
                        nop.sync_info = mybir.SyncInfo(on_wait=[w], on_update=[])
                        newinsts.append(nop)
                    ins.sync_info = mybir.SyncInfo(on_wait=[waits[-1]],
                                                   on_update=list(si.on_update))
                newinsts.append(ins)
            blk.instructions = newinsts


def make_in_maps(embedding_features, db_embedding, auxiliary_features):
    q = np.ascontiguousarray(np.asarray(embedding_features, dtype=np.float32))
    db = np.ascontiguousarray(np.asarray(db_embedding, dtype=np.float32))
    aux = np.ascontiguousarray(np.asarray(auxiliary_features, dtype=np.float32))
    ndb = db.shape[0]
    nq_core = q.shape[0] // N_CORES
    bf = ml_dtypes.bfloat16
    bias = -0.5 * (db * db).sum(1)                      # exact f32
    b_hi = bias.astype(bf).astype(np.float32)
    b_lo = (bias - b_hi).astype(bf)
    dbT_aug = np.ascontiguousarray(np.concatenate(
        [db.T.astype(bf), b_hi.astype(bf)[None, :], b_lo[None, :]], axis=0,
        dtype=bf))
    # paired table: fold slot (w, p, u) covers db rows j1 = w*win + 2p*S + u
    # and j2 = j1 + S
    win = min(WINDOW, ndb)
    n_win = ndb // win
    n_pair = win // (2 * SUPER)
    idx = np.arange(ndb // 2)
    w_i = idx // (win // 2)
    rem = idx % (win // 2)
    p_i = rem // SUPER
    u_i = rem % SUPER
    j1 = w_i * win + 2 * p_i * SUPER + u_i
    j2 = j1 + SUPER
    dbsq = (db * db).sum(1)
    pair_table = np.zeros((ndb // 2, PV), np.float32)
    pair_table[:, 0:D] = db[j1]
    pair_table[:, D:2 * D] = aux[j1]
    pair_table[:, 2 * D:3 * D] = db[j2]
    pair_table[:, 3 * D:4 * D] = aux[j2]
    pair_table[:, 4 * D] = dbsq[j1]
    pair_table[:, 4 * D + 1] = dbsq[j2]
    pair_table = np.ascontiguousarray(pair_table)
    in_maps = []
    for c in range(N_CORES):
        qs = q[c * nq_core:(c + 1) * nq_core]
        qT_aug = np.ascontiguousarray(np.concatenate(
            [qs.T.astype(bf), np.ones((2, nq_core), bf)], axis=0, dtype=bf))
        qsq = np.ascontiguousarray((qs * qs).sum(1).reshape(nq_core, 1)
                                   ).astype(np.float32)
        in_maps.append({"qT_aug": qT_aug, "qf": qs, "qsq": qsq,
                        "dbT_aug": dbT_aug, "pair_table": pair_table})
    return in_maps


_NC_CACHE = {}


def get_nc(nq_core=NQ_CORE, ndb=NDB):
    key = (nq_core, ndb)
    if key not in _NC_CACHE:
        _NC_CACHE[key] = build_nc(nq_core, ndb)
    return _NC_CACHE[key]


def kernel(embedding_features, db_embedding, auxiliary_features):
    in_maps = make_in_maps(embedding_features, db_embedding, auxiliary_features)
    nc = get_nc()
    res = run_bass_kernel_spmd(nc, in_maps, list(range(N_CORES)))
    return np.concatenate([res.results[c]["out"] for c in range(N_CORES)],
                          axis=0).astype(np.float32)
